# revision 1
# baseline (speedup 1.0000x reference)
"""Trainium2 Bass kernel for nn_AndAttention.

B=16384 rows; per row: 2-token self-attention over (x1,x2) [D=1024 each],
concat -> h [2048], then 4x (Linear(2048,2048)+ReLU) and Linear(2048,1024).

Sharding: data-parallel over batch across 8 NeuronCores (2048 rows/core),
weights replicated. No collectives.

Layout strategy (per core):
  - Activations live feature-major in SBUF: hT[feature partition, batch free].
  - 2-token softmax == sigmoid of logit differences; dot products via ACT
    Square-with-accumulate and DVE scalar_tensor_tensor-with-accumulate.
  - Attention combine+transpose fused on the PE:
      psum[d, 0:256] = x1c.T @ [diag(a00)|diag(a10)] + x2c.T @ [diag(a01)|diag(a11)]
    which yields y0^T and y1^T directly (feature-major h0).
  - x is pre-cast to bf16 and interleaved [row, token, 1024] on the host,
    so one gpsimd-ring DMA per 128-row tile fetches both tokens and the
    PE can start layer 1 by ~17us. The scalar queue carries no DMA issues
    during attention (each DMA_DIRECT2D costs ~0.65us of engine time and
    ring-window waits would block ACT compute behind it).
  - Layer 1 interleaves with attention per batch-quarter. Its psum
    evictions run on the GPSIMD engine (idle once x is loaded) so the DVE
    and ACT queues carry only attention work and the next quarter's diag
    coefficients are always ready before the PE finishes the current block.
    Emission order is [attn quarter 0][L1 block 0][attn quarter 1][L1 block
    1]... so the in-order PE queue never waits mid-block.
  - MLP layers: lhsT = pre-transposed bf16 weight tiles streamed on the
    sync ring (4 k-subtiles per DMA, 24-deep pool to prefetch through
    layer 1's 4x weight re-read), rhs = hT; psum evicted with fused
    ReLU+bias on the scalar engine.
  - Last-layer weights + bias stream on the gpsimd ring during layer 2
    (recycling the x-tile SBUF slots), keeping the sync ring clear.
  - Last layer swaps matmul args (lhsT = hT chunk, rhs = W_last^T tiles) so
    psum comes out in natural [batch, out] layout; bias added on DVE from a
    host-replicated bias tile; DMA straight to the output. The final batch
    chunk evicts in 256-col pieces so the tail DMA starts earlier.
"""

import sys

if "/opt/trn_rl_repo" not in sys.path:
    sys.path.insert(0, "/opt/trn_rl_repo")

import numpy as np
import ml_dtypes

import concourse.bass as bass
import concourse.tile as tile
from concourse import bacc, mybir
from concourse.bass_utils import run_bass_kernel_spmd
from concourse.masks import make_identity

P = 128
D = 1024
D2 = 2048
DOUT = 1024
N_LAYERS = 4
N_CORES = 8
B = 16384
BC = B // N_CORES           # rows per core = 2048
BP = BC                     # single pass over the whole core batch
NB_TILES = BC // P          # 16 b-tiles of 128 rows per core
KT = D2 // P                # 16 k tiles (contraction)
MT = D2 // P                # 16 m tiles (layer out features)
KG = 4                      # k-subtiles per weight DMA
NCHUNK = 512                # matmul moving free dim
NQ = 512                    # last-layer o-half width
QT = DOUT // NQ             # 2

f32 = mybir.dt.float32
bf16 = mybir.dt.bfloat16
NP_BF16 = np.dtype(ml_dtypes.bfloat16)
AF = mybir.ActivationFunctionType
ALU = mybir.AluOpType


def build_graph(debug_stage=None):
    nc = bacc.Bacc("TRN2", target_bir_lowering=False, debug=False,
                   num_devices=N_CORES)

    # x1/x2 interleaved host-side: one DMA per tile fetches both tokens
    xz_ext = nc.declare_dram_parameter("xz", [BC, 2, D], bf16, isOutput=False)
    # weight tiles: [l, m, kg, i(128), kk(4), o(128)] bf16 with
    #   wt[l, m, kg, i, kk, o] = Ws[l, m*128+o, (kg*4+kk)*128+i]
    wt_ext = nc.declare_dram_parameter("wt", [N_LAYERS, MT, KT // KG, P, KG, P],
                                       bf16, isOutput=False)
    # last-layer tiles: [j, i(128), r(2), o(1024)] bf16 with
    #   wlt[j,i,r,o] = W_last[o, (2j+r)*128+i]  (k pairs packed per tile)
    wlt_ext = nc.declare_dram_parameter("wlt", [KT // 2, P, 2, DOUT], bf16,
                                        isOutput=False)
    # biases: bst[l, p, m] = bs[l, m*128+p]
    bst_ext = nc.declare_dram_parameter("bst", [N_LAYERS, P, MT], f32,
                                        isOutput=False)
    # b_last replicated across partitions: [128, 1024] bf16
    blb_ext = nc.declare_dram_parameter("blb", [P, DOUT], bf16, isOutput=False)
    out_ext = nc.declare_dram_parameter("out", [BC, DOUT], f32, isOutput=True)
    dbg_ext = None
    if debug_stage is not None:
        dbg_ext = nc.declare_dram_parameter("dbg", [P, KT, BP], bf16,
                                            isOutput=True)

    with tile.TileContext(nc) as tc:
        _trace(nc, tc, xz_ext, wt_ext, wlt_ext, bst_ext, blb_ext,
               out_ext, debug_stage, dbg_ext)
    nc.compile()
    return nc


def _trace(nc, tc, xz_ext, wt_ext, wlt_ext, bst_ext, blb_ext, out_ext,
           debug_stage=None, dbg_ext=None):
    from contextlib import ExitStack
    ctx = ExitStack()
    with ctx:
        const = ctx.enter_context(tc.tile_pool(name="const", bufs=1))
        acts = ctx.enter_context(tc.tile_pool(name="acts", bufs=2))
        wpool = ctx.enter_context(tc.tile_pool(name="wpool", bufs=24))
        # 2KB-per-partition slots: rotates x tiles during attention, then the
        # 16 last-layer weight tiles land in the same slots (disjoint in time)
        xwpool = ctx.enter_context(tc.tile_pool(name="xwpool", bufs=8))
        spool = ctx.enter_context(tc.tile_pool(name="spool", bufs=2))
        stpool = ctx.enter_context(tc.tile_pool(name="stpool", bufs=4))
        smpool = ctx.enter_context(tc.tile_pool(name="smpool", bufs=4))
        dpool = ctx.enter_context(tc.tile_pool(name="dpool", bufs=3))
        mpsum = ctx.enter_context(tc.tile_pool(name="mpsum", bufs=8,
                                               space="PSUM"))

        # constants (tiles declared here; init ops emitted after the first
        # attention DMAs so the gpsimd queue issues xz tiles immediately)
        ident = const.tile([P, P], f32)
        warm = const.tile([P, 1], f32)
        bst_sb = const.tile([P, N_LAYERS * MT], f32)
        blb_sb = const.tile([P, DOUT], bf16)

        def init_consts():
            nc.vector.memset(warm[:], 0.0)
            nc.scalar.activation(warm[:], warm[:], AF.Sigmoid)
            make_identity(nc, ident)
            for l in range(N_LAYERS):
                nc.sync.dma_start(bst_sb[:, l * MT:(l + 1) * MT],
                                  bst_ext.ap()[l])

        # ---------- attention: build h0T [2048 feat, 2048 batch] ----------
        h0 = acts.tile([P, KT, BP], bf16, name="hbuf")
        xc_tiles = {}
        diag_tiles = {}

        def attn_dma(t_lo, t_hi):
            for t in range(t_lo, t_hi):
                # one DMA on the gpsimd ring per tile; the scalar queue
                # stays pure-compute and the sync ring pure-weights.
                # For the head quarter, split by token across the scalar and
                # gpsimd rings so tile 0's stats can start ~10us.
                xc = xwpool.tile([P, 2, D], bf16, name="xc")
                ap = xz_ext.ap()[t * P:(t + 1) * P, :, :]
                if t == 0:
                    # two half-DMAs: the first Square starts ~1us earlier,
                    # as soon as the x1 half lands
                    nc.gpsimd.dma_start(xc[:, 0, :], ap[:, 0, :])
                    nc.gpsimd.dma_start(xc[:, 1, :], ap[:, 1, :])
                else:
                    nc.gpsimd.dma_start(xc[:], ap)
                xc_tiles[t] = (xc[:, 0, :], xc[:, 1, :])

        def attn_stats(t):
            xc1, xc2 = xc_tiles[t]
            stat = smpool.tile([P, 4], f32, name="stat")
            # logits (already include the 1/32 temperature):
            # s11/s22 via ACT Square(x/sqrt(32)) with accumulate,
            # s12 via DVE (x1*(1/32))*x2 with accumulate
            scr = spool.tile([P, D], bf16, name="scr")
            nc.scalar.activation(scr[:], xc1[:], AF.Square,
                                 scale=float(1.0 / np.sqrt(32.0)),
                                 accum_out=stat[:, 0:1])
            scr2 = spool.tile([P, D], bf16, name="scr")
            nc.vector.scalar_tensor_tensor(scr2[:], xc1[:], 1.0 / 32.0,
                                           xc2[:], ALU.mult, ALU.mult,
                                           accum_out=stat[:, 1:2])
            scr3 = spool.tile([P, D], bf16, name="scr")
            nc.scalar.activation(scr3[:], xc2[:], AF.Square,
                                 scale=float(1.0 / np.sqrt(32.0)),
                                 accum_out=stat[:, 2:3])

            # one batched sigmoid over [d0, -d0, d1, -d1] yields
            # [a00, a01, a10, a11] in a single ACT op
            dt_ = smpool.tile([P, 4], f32, name="dt")
            nc.vector.tensor_sub(dt_[:, 0:1], stat[:, 0:1], stat[:, 1:2])
            nc.vector.tensor_sub(dt_[:, 1:2], stat[:, 1:2], stat[:, 0:1])
            nc.vector.tensor_sub(dt_[:, 2:3], stat[:, 1:2], stat[:, 2:3])
            nc.vector.tensor_sub(dt_[:, 3:4], stat[:, 2:3], stat[:, 1:2])
            coef = smpool.tile([P, 4], f32, name="coef")
            nc.scalar.activation(coef[:], dt_[:], AF.Sigmoid)

            # diagA = [diag(a00)|diag(a10)], diagB = [diag(a01)|diag(a11)]
            diagA = dpool.tile([P, 2 * P], bf16, name="diagA")
            nc.vector.tensor_scalar_mul(diagA[:, 0:P], ident[:],
                                        coef[:, 0:1])
            nc.vector.tensor_scalar_mul(diagA[:, P:2 * P], ident[:],
                                        coef[:, 2:3])
            diagB = dpool.tile([P, 2 * P], bf16, name="diagB")
            nc.vector.tensor_scalar_mul(diagB[:, 0:P], ident[:],
                                        coef[:, 1:2])
            nc.vector.tensor_scalar_mul(diagB[:, P:2 * P], ident[:],
                                        coef[:, 3:4])
            diag_tiles[t] = (diagA, diagB)

        def attn_combine(t):
            xc1, xc2 = xc_tiles.pop(t)  # noqa: kept until here for pool rotation
            diagA, diagB = diag_tiles.pop(t)
            col = t * P
            for dc in range(D // P):  # 8 feature chunks
                ps = mpsum.tile([P, NCHUNK], f32, name="mps")
                nc.tensor.matmul(ps[:, 0:2 * P],
                                 xc1[:, dc * P:(dc + 1) * P],
                                 diagA[:], start=True, stop=False)
                nc.tensor.matmul(ps[:, 0:2 * P],
                                 xc2[:, dc * P:(dc + 1) * P],
                                 diagB[:], start=False, stop=True)
                # one strided copy covers both tokens' chunks (k-slices dc
                # and dc+8); DVE strided copies run ~1.5x faster than ACT's,
                # so DVE takes 5 of the 8 chunks
                dst = h0[:, dc::8, col:col + P]
                if dc in (0, 3, 6):
                    nc.scalar.copy(dst, ps[:, 0:2 * P])
                else:
                    nc.vector.tensor_copy(dst, ps[:, 0:2 * P])

        def layer1_block(h_in, h_out, n, hooks=None, preloaded=None):
            for m in range(MT):
                ps = mpsum.tile([P, NCHUNK], f32, name="mps")
                for kg in range(KT // KG):
                    if preloaded is not None and (m, kg) in preloaded:
                        wt = preloaded.pop((m, kg))
                    else:
                        wt = wpool.tile([P, KG, P], bf16, name="wt")
                        nc.sync.dma_start(wt[:], wt_ext.ap()[0, m, kg])
                    for kk in range(KG):
                        k = kg * KG + kk
                        nc.tensor.matmul(
                            ps[:], wt[:, kk, :],
                            h_in[:, k, n * NCHUNK:(n + 1) * NCHUNK],
                            start=(k == 0), stop=(k == KT - 1))
                # evictions alternate ACT/DVE so neither queue blocks the
                # interleaved attention work of the next quarter for long
                dst = h_out[:, m, n * NCHUNK:(n + 1) * NCHUNK]
                if m % 2 == 0:
                    nc.scalar.activation(dst, ps[:], AF.Relu,
                                         bias=bst_sb[:, m:m + 1])
                else:
                    nc.vector.tensor_scalar(dst, ps[:], bst_sb[:, m:m + 1],
                                            0.0, ALU.add, ALU.max)
                if hooks and m in hooks:
                    hooks[m]()

        if debug_stage == "attn":
            attn_dma(0, NB_TILES)
            init_consts()
            for t in range(NB_TILES):
                attn_stats(t)
                attn_combine(t)
            nc.sync.dma_start(dbg_ext.ap()[:, :, :], h0[:])
            return

        # interleave: each layer-1 n-chunk only needs a quarter of the batch
        # columns; the next quarter's attention is hooked into the m-loop
        # (stats at m=1,5,9,13; PE combines at m=3,7,11,15) so its diag
        # coefficients always lead the PE and no in-order queue blocks.
        # The first two m-tiles' weights are issued before the rest so
        # their data beats h0 readiness on the slow-ramping sync ring.
        attn_dma(0, 4)
        init_consts()
        # dummy matmul stream keeps the PE continuously busy from ~7us so
        # the clock p-state is fully ramped when the real combines arrive
        # (cold-start matmuls otherwise run ~1.4x slow for the first ~3us)
        wsrc = const.tile([P, NCHUNK], bf16)
        nc.vector.memset(wsrc[:], 0.0)
        wps = mpsum.tile([P, NCHUNK], f32, name="mps")
        for _ in range(45):
            nc.tensor.matmul(wps[:], wsrc[:, 0:P], wsrc[:],
                             start=True, stop=True)
        preloaded = {}
        for m in range(2):
            for kg in range(KT // KG):
                wt = wpool.tile([P, KG, P], bf16, name="wt")
                nc.sync.dma_start(wt[:], wt_ext.ap()[0, m, kg])
                preloaded[(m, kg)] = wt
        for t in range(4):
            attn_stats(t)
            attn_combine(t)
        h1 = acts.tile([P, KT, BP], bf16, name="hbuf")
        for n in range(4):
            hooks = None
            if n < 3:
                attn_dma(4 * (n + 1), 4 * (n + 2))
                base = 4 * (n + 1)
                hooks = {}
                for i in range(4):
                    hooks[4 * i + 1] = (lambda t=base + i: attn_stats(t))
                    hooks[4 * i + 3] = (lambda t=base + i: attn_combine(t))
            layer1_block(h0, h1, n, hooks,
                         preloaded=preloaded if n == 0 else None)
        h = h1

        # ---------- MLP layers 2..4 (feature-major) ----------
        wl_tiles = []
        for l in range(1, N_LAYERS):
            if l == 2:
                # last-layer weights + bias stream on the (now idle) gpsimd
                # ring into the recycled x-tile slots
                nc.gpsimd.dma_start(blb_sb[:], blb_ext.ap()[:, :])
                for j in range(KT // 2):
                    wl = xwpool.tile([P, 2, DOUT], bf16, name="xc")
                    nc.gpsimd.dma_start(wl[:], wlt_ext.ap()[j])
                    wl_tiles.append(wl)
            hout = acts.tile([P, KT, BP], bf16, name="hbuf")
            for m in range(MT):
                pss = [mpsum.tile([P, NCHUNK], f32, name="mps")
                       for _ in range(BP // NCHUNK)]
                for kg in range(KT // KG):
                    wt = wpool.tile([P, KG, P], bf16, name="wt")
                    nc.sync.dma_start(wt[:], wt_ext.ap()[l, m, kg])
                    for kk in range(KG):
                        k = kg * KG + kk
                        first = (k == 0)
                        last = (k == KT - 1)
                        for nn in range(BP // NCHUNK):
                            nc.tensor.matmul(
                                pss[nn][:], wt[:, kk, :],
                                h[:, k, nn * NCHUNK:(nn + 1) * NCHUNK],
                                start=first, stop=last)
                bias = bst_sb[:, l * MT + m:l * MT + m + 1]
                for nn in range(BP // NCHUNK):
                    nc.scalar.activation(hout[:, m, nn * NCHUNK:(nn + 1) * NCHUNK],
                                         pss[nn][:], AF.Relu, bias=bias)
            h = hout

        if debug_stage == "mlp":
            nc.sync.dma_start(dbg_ext.ap()[:, :, :], h[:])
            return

        # ---------- last layer: natural-layout output ----------
        for m in range(BP // P):  # 16 batch chunks of 128
            pss = [mpsum.tile([P, NCHUNK], f32, name="mps")
                   for _ in range(QT)]
            for k in range(KT):
                for q in range(QT):
                    nc.tensor.matmul(pss[q][:], h[:, k, m * P:(m + 1) * P],
                                     wl_tiles[k // 2][:, k % 2, q * NQ:(q + 1) * NQ],
                                     start=(k == 0), stop=(k == KT - 1))
            r0 = m * P
            for q in range(QT):
                stg = stpool.tile([P, NQ], f32, name="stg")
                nc.vector.tensor_add(stg[:], pss[q][:],
                                     blb_sb[:, q * NQ:(q + 1) * NQ])
                # final chunk: q1 rides the idle gpsimd ring so the two
                # tail DMAs' data transfers overlap instead of serializing
                eng = nc.gpsimd if (m == BP // P - 1 and q == 1) else nc.sync
                eng.dma_start(
                    out_ext.ap()[r0:r0 + P, q * NQ:(q + 1) * NQ], stg[:])


def prep_inputs(x1, x2, Ws, bs, W_last, b_last):
    """Host-side layout prep shared by all cores (weights) + per-core shards."""
    wt = np.ascontiguousarray(
        Ws.reshape(N_LAYERS, MT, P, KT // KG, KG, P)
        .transpose(0, 1, 3, 5, 4, 2)).astype(NP_BF16)
    wlt = np.ascontiguousarray(
        W_last.reshape(DOUT, KT // 2, 2, P).transpose(1, 3, 2, 0)).astype(NP_BF16)
    bst = np.ascontiguousarray(
        bs.reshape(N_LAYERS, MT, P).transpose(0, 2, 1))
    blb = np.ascontiguousarray(
        np.broadcast_to(b_last, (P, DOUT))).astype(NP_BF16)
    xz = np.ascontiguousarray(
        np.stack([x1, x2], axis=1)).astype(NP_BF16)
    shared = {"wt": wt, "wlt": wlt, "bst": bst, "blb": blb}
    in_maps = []
    for c in range(N_CORES):
        sl = slice(c * BC, (c + 1) * BC)
        m = {"xz": np.ascontiguousarray(xz[sl])}
        m.update(shared)
        in_maps.append(m)
    return in_maps


_compiled_nc = None


def kernel(x1, x2, Ws, bs, W_last, b_last):
    global _compiled_nc
    x1 = np.asarray(x1, dtype=np.float32)
    x2 = np.asarray(x2, dtype=np.float32)
    Ws = np.asarray(Ws, dtype=np.float32)
    bs = np.asarray(bs, dtype=np.float32)
    W_last = np.asarray(W_last, dtype=np.float32)
    b_last = np.asarray(b_last, dtype=np.float32)

    if _compiled_nc is None:
        _compiled_nc = build_graph()
    in_maps = prep_inputs(x1, x2, Ws, bs, W_last, b_last)
    res = run_bass_kernel_spmd(_compiled_nc, in_maps,
                               core_ids=list(range(N_CORES)))
    out = np.concatenate([res.results[c]["out"] for c in range(N_CORES)],
                         axis=0)
    return out.astype(np.float32)



# revision 2
# speedup vs baseline: 1.0017x; 1.0017x over previous
"""Trainium2 Bass kernel for nn_AndAttention — v5.

B=16384 rows; per row: 2-token self-attention over (x1,x2) [D=1024 each],
concat -> h [2048], then 4x (Linear(2048,2048)+ReLU) and Linear(2048,1024).

Sharding: data-parallel over batch across 8 NeuronCores (2048 rows/core),
weights replicated. No collectives.

The kernel is matmul-stream-bound: the PE runs back-to-back N=512 bf16
matmuls at the 213ns hardware floor for ~99% of the span. v5 shortens the
stream and keeps the proven v1 startup:

  - tiles 0-3 (startup): v1's fused combine+transpose on the PE
    (psum = x1c.T@[diag(a00)|diag(a10)] + x2c.T@[diag(a01)|diag(a11)]),
    because the PE is idle waiting for the first x tiles anyway and the
    PE combine has the shortest dependency chain -> layer 1 starts ~24us.
    Their x tiles ride the sync HWDGE ring (lands ~3us earlier than SWDGE).
  - tiles 4-15 (steady state): the combine runs on the DVE instead
    (y0 = x2 + a00*(x1-x2), softmax rows sum to 1), using host-pre-
    transposed feature-major x column blocks; per-row coefficients are
    partition-broadcast by ONE tiny N=256 PE matmul per tile
    (psum = ones.T @ [diag(a00)|diag(a10)]). This removes 192 N=256
    matmuls (~20us) from the PE stream where it is the bottleneck.
  - stats: s11 via ACT Square-with-accum, s12/s22 via DVE
    scalar_tensor_tensor-with-accum; sigmoid on ACT. Sigmoid+Square ACT
    table sets are warmed before the first x tile lands.
  - MLP layers: lhsT = pre-transposed bf16 weight tiles streamed on the
    sync ring, rhs = hT; psum evicted with fused ReLU+bias on ACT (and DVE
    for layer 1's odd m, to keep both queues short during the interleave).
  - Last layer swaps matmul args so psum comes out in natural [batch, out]
    layout; the final batch chunk evicts in 256-col pieces across both
    rings so the tail DMAs overlap.
"""

import sys

if "/opt/trn_rl_repo" not in sys.path:
    sys.path.insert(0, "/opt/trn_rl_repo")

import numpy as np
import ml_dtypes

import concourse.bass as bass
import concourse.tile as tile
from concourse import bacc, mybir
from concourse.bass_utils import run_bass_kernel_spmd
from concourse.masks import make_identity

P = 128
D = 1024
D2 = 2048
DOUT = 1024
N_LAYERS = 4
N_CORES = 8
B = 16384
BC = B // N_CORES           # rows per core = 2048
BP = BC                     # single pass over the whole core batch
NB_TILES = BC // P          # 16 b-tiles of 128 rows per core
KT = D2 // P                # 16 k tiles (contraction)
MT = D2 // P                # 16 m tiles (layer out features)
KG = 4                      # k-subtiles per weight DMA
NCHUNK = 512                # matmul moving free dim
NQ = 512                    # last-layer o-half width
QT = DOUT // NQ             # 2
DC = D // P                 # 8 feature chunks per token

WARM0 = 42                  # N=512 dummy matmuls bridging PE to first combine

f32 = mybir.dt.float32
bf16 = mybir.dt.bfloat16
NP_BF16 = np.dtype(ml_dtypes.bfloat16)
AF = mybir.ActivationFunctionType
ALU = mybir.AluOpType


def build_graph(debug_stage=None):
    nc = bacc.Bacc("TRN2", target_bir_lowering=False, debug=False,
                   num_devices=N_CORES)

    # x1/x2 interleaved host-side (batch-major, for stats + startup combine)
    xz_ext = nc.declare_dram_parameter("xz", [BC, 2, D], bf16, isOutput=False)
    # feature-major per-tile column blocks (steady-state combine):
    #   xzt[t, p, tok, c, j] = x_tok[t*128 + j, c*128 + p]
    xzt_ext = nc.declare_dram_parameter("xzt", [NB_TILES, P, 2, DC, P], bf16,
                                        isOutput=False)
    # weight tiles: wt[l, m, kg, i, kk, o] = Ws[l, m*128+o, (kg*4+kk)*128+i]
    wt_ext = nc.declare_dram_parameter("wt", [N_LAYERS, MT, KT // KG, P, KG, P],
                                       bf16, isOutput=False)
    # last-layer tiles: wlt[j,i,r,o] = W_last[o, (2j+r)*128+i]
    wlt_ext = nc.declare_dram_parameter("wlt", [KT // 2, P, 2, DOUT], bf16,
                                        isOutput=False)
    # biases: bst[l, p, m] = bs[l, m*128+p]
    bst_ext = nc.declare_dram_parameter("bst", [N_LAYERS, P, MT], f32,
                                        isOutput=False)
    # b_last replicated across partitions: [128, 1024] bf16
    blb_ext = nc.declare_dram_parameter("blb", [P, DOUT], bf16, isOutput=False)
    out_ext = nc.declare_dram_parameter("out", [BC, DOUT], f32, isOutput=True)
    dbg_ext = None
    if debug_stage is not None:
        dbg_ext = nc.declare_dram_parameter("dbg", [P, KT, BP], bf16,
                                            isOutput=True)

    with tile.TileContext(nc) as tc:
        _trace(nc, tc, xz_ext, xzt_ext, wt_ext, wlt_ext, bst_ext, blb_ext,
               out_ext, debug_stage, dbg_ext)
    nc.compile()
    return nc


def _trace(nc, tc, xz_ext, xzt_ext, wt_ext, wlt_ext, bst_ext, blb_ext,
           out_ext, debug_stage=None, dbg_ext=None):
    from contextlib import ExitStack
    ctx = ExitStack()
    with ctx:
        const = ctx.enter_context(tc.tile_pool(name="const", bufs=1))
        acts = ctx.enter_context(tc.tile_pool(name="acts", bufs=2))
        wpool = ctx.enter_context(tc.tile_pool(name="wpool", bufs=14))
        # batch-major x tiles for steady-state stats (4KB slots)
        xzpool = ctx.enter_context(tc.tile_pool(name="xzpool", bufs=3))
        # 4KB slots: startup x tiles 0-3, then feature-major xzt tiles 4-15,
        # then the 8 last-layer weight tiles (all disjoint in time)
        xztpool = ctx.enter_context(tc.tile_pool(name="xztpool", bufs=8))
        spool = ctx.enter_context(tc.tile_pool(name="spool", bufs=2))
        stpool = ctx.enter_context(tc.tile_pool(name="stpool", bufs=2))
        smpool = ctx.enter_context(tc.tile_pool(name="smpool", bufs=6))
        dpool = ctx.enter_context(tc.tile_pool(name="dpool", bufs=3))
        abcpool = ctx.enter_context(tc.tile_pool(name="abcpool", bufs=2))
        cpool = ctx.enter_context(tc.tile_pool(name="cpool", bufs=3))
        mpsum = ctx.enter_context(tc.tile_pool(name="mpsum", bufs=8,
                                               space="PSUM"))

        ident = const.tile([P, P], f32)
        ones_bf = const.tile([P, P], bf16)
        warm = const.tile([P, 1], f32)
        bst_sb = const.tile([P, N_LAYERS * MT], f32)
        blb_sb = const.tile([P, DOUT], bf16)
        # warmup dummies read blb_sb's (not-yet-loaded) bytes as garbage bf16;
        # the real b_last DMA lands only at layer 2, long after warmup
        wsrc = blb_sb

        def init_consts():
            nc.vector.memset(warm[:], 0.0)
            nc.scalar.activation(warm[:], warm[:], AF.Sigmoid)
            nc.scalar.activation(warm[:], warm[:], AF.Square)
            nc.vector.memset(ones_bf[:], 1.0)
            make_identity(nc, ident)
            for l in range(N_LAYERS):
                nc.sync.dma_start(bst_sb[:, l * MT:(l + 1) * MT],
                                  bst_ext.ap()[l])

        # ---------- attention: build h0T [2048 feat, 2048 batch] ----------
        h0 = acts.tile([P, KT, BP], bf16, name="hbuf")
        xz_tiles = {}
        xzt_tiles = {}
        coef_tiles = {}
        abc_tiles = {}
        diag_tiles = {}

        def xz_dma(t_lo, t_hi):
            for t in range(t_lo, t_hi):
                if t < 4:
                    # startup tiles: sync HWDGE ring (fast first byte), into
                    # the xzt pool slots (free until tile 4's xzt arrives)
                    xc = xztpool.tile([P, 2, D], bf16, name="xzt")
                    nc.sync.dma_start(xc[:], xz_ext.ap()[t * P:(t + 1) * P])
                else:
                    xc = xzpool.tile([P, 2, D], bf16, name="xc")
                    nc.gpsimd.dma_start(xc[:], xz_ext.ap()[t * P:(t + 1) * P])
                xz_tiles[t] = xc

        def xzt_dma(t_lo, t_hi):
            for t in range(t_lo, t_hi):
                xt = xztpool.tile([P, 2, DC, P], bf16, name="xzt")
                nc.sync.dma_start(xt[:], xzt_ext.ap()[t])
                xzt_tiles[t] = xt

        def attn_stats(t, four_coef):
            xc = xz_tiles[t] if four_coef else xz_tiles.pop(t)
            xc1, xc2 = xc[:, 0, :], xc[:, 1, :]
            stat = smpool.tile([P, 4], f32, name="stat")
            scr = spool.tile([P, D], bf16, name="scr")
            nc.scalar.activation(scr[:], xc1[:], AF.Square,
                                 scale=float(1.0 / np.sqrt(32.0)),
                                 accum_out=stat[:, 0:1])
            scr2 = spool.tile([P, D], bf16, name="scr")
            nc.vector.scalar_tensor_tensor(scr2[:], xc1[:], 1.0 / 32.0,
                                           xc2[:], ALU.mult, ALU.mult,
                                           accum_out=stat[:, 1:2])
            if four_coef:
                # startup: balance the two squares on ACT with the stt on
                # DVE so neither engine serializes the 4-tile chain
                nc.scalar.activation(scr[:], xc2[:], AF.Square,
                                     scale=float(1.0 / np.sqrt(32.0)),
                                     accum_out=stat[:, 2:3])
            else:
                nc.vector.scalar_tensor_tensor(scr2[:], xc2[:], 1.0 / 32.0,
                                               xc2[:], ALU.mult, ALU.mult,
                                               accum_out=stat[:, 2:3])
            if four_coef:
                # [a00, a01, a10, a11] in one batched sigmoid
                dt_ = smpool.tile([P, 4], f32, name="dt")
                nc.vector.tensor_sub(dt_[:, 0:1], stat[:, 0:1], stat[:, 1:2])
                nc.vector.tensor_sub(dt_[:, 1:2], stat[:, 1:2], stat[:, 0:1])
                nc.vector.tensor_sub(dt_[:, 2:3], stat[:, 1:2], stat[:, 2:3])
                nc.vector.tensor_sub(dt_[:, 3:4], stat[:, 2:3], stat[:, 1:2])
                coef = smpool.tile([P, 4], f32, name="coef")
                nc.scalar.activation(coef[:], dt_[:], AF.Sigmoid)
            else:
                dt_ = smpool.tile([P, 2], f32, name="dt")
                nc.vector.tensor_sub(dt_[:, 0:1], stat[:, 0:1], stat[:, 1:2])
                nc.vector.tensor_sub(dt_[:, 1:2], stat[:, 1:2], stat[:, 2:3])
                coef = smpool.tile([P, 2], f32, name="coef")
                nc.scalar.activation(coef[:], dt_[:], AF.Sigmoid)
            coef_tiles[t] = coef

        # --- startup path (tiles 0-3): v1 fused combine+transpose on PE ---
        def attn_diag4(t):
            coef = coef_tiles.pop(t)
            diagA = dpool.tile([P, 2 * P], bf16, name="diagA")
            nc.vector.tensor_scalar_mul(diagA[:, 0:P], ident[:], coef[:, 0:1])
            nc.vector.tensor_scalar_mul(diagA[:, P:2 * P], ident[:],
                                        coef[:, 2:3])
            diagB = dpool.tile([P, 2 * P], bf16, name="diagB")
            nc.vector.tensor_scalar_mul(diagB[:, 0:P], ident[:], coef[:, 1:2])
            nc.vector.tensor_scalar_mul(diagB[:, P:2 * P], ident[:],
                                        coef[:, 3:4])
            diag_tiles[t] = (diagA, diagB)

        def attn_combine_pe(t):
            xc = xz_tiles.pop(t)
            xc1, xc2 = xc[:, 0, :], xc[:, 1, :]
            diagA, diagB = diag_tiles.pop(t)
            col = t * P
            for dc in range(DC):
                ps = mpsum.tile([P, NCHUNK], f32, name="mps")
                nc.tensor.matmul(ps[:, 0:2 * P],
                                 xc1[:, dc * P:(dc + 1) * P],
                                 diagA[:], start=True, stop=False)
                nc.tensor.matmul(ps[:, 0:2 * P],
                                 xc2[:, dc * P:(dc + 1) * P],
                                 diagB[:], start=False, stop=True)
                dst = h0[:, dc::DC, col:col + P]
                if dc in (0, 3, 6):
                    nc.scalar.copy(dst, ps[:, 0:2 * P])
                else:
                    nc.vector.tensor_copy(dst, ps[:, 0:2 * P])

        # --- steady-state path (tiles 4-15): DVE combine ---
        def attn_bcast(t):
            coef = coef_tiles.pop(t)
            diag = dpool.tile([P, 2, P], bf16, name="diagA")
            nc.vector.tensor_scalar_mul(diag[:, 0, :], ident[:], coef[:, 0:1])
            nc.vector.tensor_scalar_mul(diag[:, 1, :], ident[:], coef[:, 1:2])
            ps = mpsum.tile([P, NCHUNK], f32, name="mps")
            nc.tensor.matmul(ps[:, 0:2 * P], ones_bf[:], diag[:, :, :],
                             start=True, stop=True)
            abc = abcpool.tile([P, 2, P], bf16, name="abc")
            nc.scalar.copy(abc[:], ps[:, 0:2 * P])
            abc_tiles[t] = abc

        def attn_combine(t):
            xt = xzt_tiles.pop(t)
            abc = abc_tiles.pop(t)
            x1T, x2T = xt[:, 0], xt[:, 1]      # [P, DC, P]
            col = t * P
            dif = cpool.tile([P, DC, P], bf16, name="dif")
            nc.vector.tensor_sub(dif[:], x1T[:], x2T[:])
            a00 = abc[:, 0:1, :].broadcast_to([P, DC, P])
            a10 = abc[:, 1:2, :].broadcast_to([P, DC, P])
            t0 = cpool.tile([P, DC, P], bf16, name="dif")
            nc.vector.tensor_mul(t0[:], dif[:], a00)
            nc.vector.tensor_add(h0[:, 0:DC, col:col + P], t0[:], x2T[:])
            t1 = cpool.tile([P, DC, P], bf16, name="dif")
            nc.vector.tensor_mul(t1[:], dif[:], a10)
            nc.vector.tensor_add(h0[:, DC:2 * DC, col:col + P], t1[:], x2T[:])

        def warmup(n):
            for _ in range(n):
                wps = mpsum.tile([P, NCHUNK], f32, name="mps")
                nc.tensor.matmul(wps[:], wsrc[:, 0:P],
                                 wsrc[:, 0:NCHUNK], start=True, stop=True)

        def layer1_block(h_in, h_out, n, hooks=None, preloaded=None):
            for m in range(MT):
                ps = mpsum.tile([P, NCHUNK], f32, name="mps")
                for kg in range(KT // KG):
                    if preloaded is not None and (m, kg) in preloaded:
                        wt = preloaded.pop((m, kg))
                    else:
                        wt = wpool.tile([P, KG, P], bf16, name="wt")
                        nc.sync.dma_start(wt[:], wt_ext.ap()[0, m, kg])
                    for kk in range(KG):
                        k = kg * KG + kk
                        nc.tensor.matmul(
                            ps[:], wt[:, kk, :],
                            h_in[:, k, n * NCHUNK:(n + 1) * NCHUNK],
                            start=(k == 0), stop=(k == KT - 1))
                dst = h_out[:, m, n * NCHUNK:(n + 1) * NCHUNK]
                if m % 2 == 0:
                    nc.scalar.activation(dst, ps[:], AF.Relu,
                                         bias=bst_sb[:, m:m + 1])
                else:
                    nc.vector.tensor_scalar(dst, ps[:], bst_sb[:, m:m + 1],
                                            0.0, ALU.add, ALU.max)
                if hooks and m in hooks:
                    hooks[m]()

        if debug_stage == "attn":
            xz_dma(0, NB_TILES)
            xzt_dma(4, NB_TILES)
            init_consts()
            for t in range(4):
                attn_stats(t, True)
                attn_diag4(t)
                attn_combine_pe(t)
            for t in range(4, NB_TILES):
                attn_stats(t, False)
                attn_bcast(t)
                attn_combine(t)
            nc.sync.dma_start(dbg_ext.ap()[:, :, :], h0[:])
            return

        # startup: x tiles 0-3 on the sync ring, then constants, then the
        # dummy matmul stream that keeps the PE clock ramped until the
        # first real combine lands.
        xz_dma(0, 4)
        init_consts()
        warmup(WARM0)
        for t in range(4):
            attn_stats(t, True)
            attn_diag4(t)
            attn_combine_pe(t)
        preloaded = {}
        for m in range(2):
            for kg in range(KT // KG):
                wt = wpool.tile([P, KG, P], bf16, name="wt")
                nc.sync.dma_start(wt[:], wt_ext.ap()[0, m, kg])
                preloaded[(m, kg)] = wt
        h1 = acts.tile([P, KT, BP], bf16, name="hbuf")
        for n in range(4):
            hooks = None
            if n < 3:
                base = 4 * (n + 1)
                xz_dma(base, base + 4)
                xzt_dma(base, base + 4)
                hooks = {}
                for i in range(4):
                    hooks[3 * i] = (lambda t=base + i: attn_stats(t, False))
                    hooks[3 * i + 1] = (lambda t=base + i: attn_bcast(t))
                    hooks[3 * i + 2] = (lambda t=base + i: attn_combine(t))
            layer1_block(h0, h1, n, hooks,
                         preloaded=preloaded if n == 0 else None)
        h = h1

        # ---------- MLP layers 2..4 (feature-major) ----------
        wl_tiles = []
        for l in range(1, N_LAYERS):
            if l == 2:
                # last-layer weights + bias stream on the (now idle) gpsimd
                # ring into the recycled xzt slots
                nc.gpsimd.dma_start(blb_sb[:], blb_ext.ap()[:, :])
                for j in range(KT // 2):
                    wl = xztpool.tile([P, 2, DOUT], bf16, name="xzt")
                    nc.gpsimd.dma_start(wl[:], wlt_ext.ap()[j])
                    wl_tiles.append(wl)
            hout = acts.tile([P, KT, BP], bf16, name="hbuf")
            for m in range(MT):
                pss = [mpsum.tile([P, NCHUNK], f32, name="mps")
                       for _ in range(BP // NCHUNK)]
                for kg in range(KT // KG):
                    wt = wpool.tile([P, KG, P], bf16, name="wt")
                    nc.sync.dma_start(wt[:], wt_ext.ap()[l, m, kg])
                    for kk in range(KG):
                        k = kg * KG + kk
                        first = (k == 0)
                        last = (k == KT - 1)
                        for nn in range(BP // NCHUNK):
                            nc.tensor.matmul(
                                pss[nn][:], wt[:, kk, :],
                                h[:, k, nn * NCHUNK:(nn + 1) * NCHUNK],
                                start=first, stop=last)
                bias = bst_sb[:, l * MT + m:l * MT + m + 1]
                for nn in range(BP // NCHUNK):
                    nc.scalar.activation(hout[:, m, nn * NCHUNK:(nn + 1) * NCHUNK],
                                         pss[nn][:], AF.Relu, bias=bias)
            h = hout

        if debug_stage == "mlp":
            nc.sync.dma_start(dbg_ext.ap()[:, :, :], h[:])
            return

        # ---------- last layer: natural-layout output ----------
        for m in range(BP // P):  # 16 batch chunks of 128
            pss = [mpsum.tile([P, NCHUNK], f32, name="mps")
                   for _ in range(QT)]
            for k in range(KT):
                for q in range(QT):
                    nc.tensor.matmul(pss[q][:], h[:, k, m * P:(m + 1) * P],
                                     wl_tiles[k // 2][:, k % 2, q * NQ:(q + 1) * NQ],
                                     start=(k == 0), stop=(k == KT - 1))
            r0 = m * P
            last_chunk = (m == BP // P - 1)
            for q in range(QT):
                stg = stpool.tile([P, NQ], f32, name="stg")
                if last_chunk:
                    # final chunk: evict in 256-col halves, alternating rings,
                    # so the tail DMAs start earlier and overlap
                    for hh in range(2):
                        sl = slice(hh * 256, (hh + 1) * 256)
                        nc.vector.tensor_add(stg[:, sl], pss[q][:, sl],
                                             blb_sb[:, q * NQ + hh * 256:
                                                    q * NQ + (hh + 1) * 256])
                        eng = nc.gpsimd if (q + hh) % 2 else nc.sync
                        eng.dma_start(
                            out_ext.ap()[r0:r0 + P,
                                         q * NQ + hh * 256:q * NQ + (hh + 1) * 256],
                            stg[:, sl])
                else:
                    nc.vector.tensor_add(stg[:], pss[q][:],
                                         blb_sb[:, q * NQ:(q + 1) * NQ])
                    nc.sync.dma_start(
                        out_ext.ap()[r0:r0 + P, q * NQ:(q + 1) * NQ], stg[:])


def prep_inputs(x1, x2, Ws, bs, W_last, b_last):
    """Host-side layout prep shared by all cores (weights) + per-core shards."""
    wt = np.ascontiguousarray(
        Ws.reshape(N_LAYERS, MT, P, KT // KG, KG, P)
        .transpose(0, 1, 3, 5, 4, 2)).astype(NP_BF16)
    wlt = np.ascontiguousarray(
        W_last.reshape(DOUT, KT // 2, 2, P).transpose(1, 3, 2, 0)).astype(NP_BF16)
    bst = np.ascontiguousarray(
        bs.reshape(N_LAYERS, MT, P).transpose(0, 2, 1))
    blb = np.ascontiguousarray(
        np.broadcast_to(b_last, (P, DOUT))).astype(NP_BF16)
    xz = np.stack([x1, x2], axis=1).astype(NP_BF16)      # [B, 2, D]
    shared = {"wt": wt, "wlt": wlt, "bst": bst, "blb": blb}
    in_maps = []
    for c in range(N_CORES):
        sl = slice(c * BC, (c + 1) * BC)
        xzc = xz[sl]                                     # [BC, 2, D]
        a = xzc.reshape(NB_TILES, P, 2, DC, P)           # [t, j, tok, c, p]
        xzt = np.ascontiguousarray(a.transpose(0, 4, 2, 3, 1))
        m = {"xz": np.ascontiguousarray(xzc), "xzt": xzt}
        m.update(shared)
        in_maps.append(m)
    return in_maps


_compiled_nc = None


def kernel(x1, x2, Ws, bs, W_last, b_last):
    global _compiled_nc
    x1 = np.asarray(x1, dtype=np.float32)
    x2 = np.asarray(x2, dtype=np.float32)
    Ws = np.asarray(Ws, dtype=np.float32)
    bs = np.asarray(bs, dtype=np.float32)
    W_last = np.asarray(W_last, dtype=np.float32)
    b_last = np.asarray(b_last, dtype=np.float32)

    if _compiled_nc is None:
        _compiled_nc = build_graph()
    in_maps = prep_inputs(x1, x2, Ws, bs, W_last, b_last)
    res = run_bass_kernel_spmd(_compiled_nc, in_maps,
                               core_ids=list(range(N_CORES)))
    out = np.concatenate([res.results[c]["out"] for c in range(N_CORES)],
                         axis=0)
    return out.astype(np.float32)
